# Initial kernel scaffold
#
"""FP32 -> FP8 E4M3 bit-pulse converter on 8 Trainium2 NeuronCores.

Input : fp32_pulse [2097152, 32] float32 of 0/1 pulses, [S, E7..E0, M22..M0]
Output: [2097152, 8] float32 of 0/1 pulses, [S, E3..E0, M2..M0]

Strategy (per core, batch-sharded 8 ways):
  - SWDGE cast-DMA loads the fp32 pulses as uint8 (4x less SBUF traffic).
  - q = 32*exp + 16*m22 + 8*m21 + 4*m20 + 2*m19 + sticky  (13-bit int) via a
    scalar_tensor_tensor MAC chain; sticky = OR of the 19 low mantissa bits,
    computed as a segmented reduce_max over the int32-bitcast byte words.
  - v = bitcast(int32(max(q,3712) * 2^18)): exactly the fp32 with exponent
    field = exp, mantissa = m22..m19 | sticky<<18.  The underflow clamp maps
    exp<=116 to a value that rounds to zero in fp8.
  - Hardware fp32->fp8e4 copy performs the exact RNE (incl. subnormals).
  - Overflow (exp>=135) forces byte 0x7E (=15/6) per the reference.
  - 7 low bits of the byte + the sign pulse are scattered to the output.
"""

import numpy as np

import concourse.bass as bass
import concourse.bacc as bacc
import concourse.mybir as mybir
from concourse import tile
from concourse.bass_utils import run_bass_kernel_spmd

N_ROWS = 2097152
N_CORES = 8
ROWS_PER_CORE = N_ROWS // N_CORES          # 262144

F8 = 8192                                   # uint8 tile bytes per partition/chunk
ROWS_PER_CHUNK = 128 * F8 // 32             # 32768
N_CHUNKS = ROWS_PER_CORE // ROWS_PER_CHUNK  # 8
SEG = F8 // 32                              # segments (rows) per partition/chunk
FO = SEG * 8                                # out floats per partition/chunk

dt = mybir.dt
Alu = mybir.AluOpType

MAC_W = [float(2 ** (13 - i)) for i in range(1, 13)]  # col i weight, i=1..12


def _build_program():
    nc = bacc.Bacc("TRN2", target_bir_lowering=False, debug=False,
                   num_devices=N_CORES)
    x_dram = nc.dram_tensor("x", [ROWS_PER_CORE, 32], dt.float32,
                            kind="ExternalInput")
    y_dram = nc.dram_tensor("y", [ROWS_PER_CORE, 8], dt.float32,
                            kind="ExternalOutput")
    x_ap = x_dram.ap().rearrange("(c p f) w -> c p (f w)", c=N_CHUNKS, p=128)
    y_ap = y_dram.ap().rearrange("(c p f) w -> c p (f w)", c=N_CHUNKS, p=128)

    with tile.TileContext(nc) as tc:
        with (
            tc.tile_pool(name="xin", bufs=3) as xin_pool,
            tc.tile_pool(name="out", bufs=3) as out_pool,
            tc.tile_pool(name="wrk", bufs=2) as wrk,
        ):
            for c in range(N_CHUNKS):
                x8 = xin_pool.tile([128, F8], dt.uint8, tag="x8")
                nc.gpsimd.dma_start(x8[:], x_ap[c])

                x8v = x8[:].rearrange("p (s c) -> p s c", c=32)
                xw3d = x8[:].bitcast(dt.int32).rearrange("p (s w) -> p s w", w=8)

                # sticky = OR of cols 13..31 = (max(words 4..7) | (word3>=256)) > 0
                red = wrk.tile([128, SEG], dt.float32, tag="red")
                nc.vector.tensor_reduce(red[:], xw3d[:, :, 4:8],
                                        axis=mybir.AxisListType.X, op=Alu.max)
                sor = wrk.tile([128, SEG], dt.float32, tag="sor")
                nc.vector.scalar_tensor_tensor(sor[:], xw3d[:, :, 3], 256.0,
                                               red[:], op0=Alu.is_ge, op1=Alu.max)

                # q MAC chain over cols 1..12, then fold sticky
                qa = wrk.tile([128, SEG], dt.float32, tag="qa")
                qb = wrk.tile([128, SEG], dt.float32, tag="qb")
                nc.vector.tensor_scalar(qa[:], x8v[:, :, 1], MAC_W[0], None,
                                        op0=Alu.mult)
                cur, nxt = qa, qb
                for i in range(2, 13):
                    nc.vector.scalar_tensor_tensor(nxt[:], x8v[:, :, i],
                                                   MAC_W[i - 1], cur[:],
                                                   op0=Alu.mult, op1=Alu.add)
                    cur, nxt = nxt, cur
                q = nxt
                nc.vector.scalar_tensor_tensor(q[:], sor[:], 0.0, cur[:],
                                               op0=Alu.is_gt, op1=Alu.add)

                # v bits = int32(max(q, 3712) * 2^18); fp8 cast on ACT
                vb = wrk.tile([128, SEG], dt.int32, tag="vb")
                nc.vector.tensor_scalar(vb[:], q[:], 3712.0, 262144.0,
                                        op0=Alu.max, op1=Alu.mult)
                f8 = wrk.tile([128, SEG], dt.float8e4, tag="f8")
                nc.scalar.copy(f8[:], vb[:].bitcast(dt.float32))
                u = wrk.tile([128, SEG], dt.float32, tag="u")
                nc.scalar.copy(u[:], f8[:].bitcast(dt.uint8))

                # overflow select: uf = u + (q>=4320)*(126-u)
                ovf = wrk.tile([128, SEG], dt.float32, tag="ovf")
                nc.gpsimd.tensor_scalar(ovf[:], q[:], 4320.0, None, op0=Alu.is_ge)
                d_t = wrk.tile([128, SEG], dt.float32, tag="d")
                nc.gpsimd.tensor_scalar(d_t[:], u[:], -1.0, 126.0,
                                        op0=Alu.mult, op1=Alu.add)
                m_t = wrk.tile([128, SEG], dt.float32, tag="m")
                nc.gpsimd.tensor_tensor(m_t[:], d_t[:], ovf[:], op=Alu.mult)
                uf = wrk.tile([128, SEG], dt.float32, tag="uf")
                nc.gpsimd.tensor_tensor(uf[:], m_t[:], u[:], op=Alu.add)
                ui = wrk.tile([128, SEG], dt.int32, tag="ui")
                nc.scalar.copy(ui[:], uf[:])

                o_t = out_pool.tile([128, FO], dt.float32, tag="o")
                o3d = o_t[:].rearrange("p (s c) -> p s c", c=8)
                nc.scalar.copy(o3d[:, :, 0], x8v[:, :, 0])       # sign
                for j in range(1, 8):
                    nc.gpsimd.tensor_scalar(o3d[:, :, j], ui[:], 7 - j, 1,
                                            op0=Alu.logical_shift_right,
                                            op1=Alu.bitwise_and)

                nc.sync.dma_start(y_ap[c], o_t[:])

    nc.compile()
    return nc


_NC_CACHE = None


def _get_nc():
    global _NC_CACHE
    if _NC_CACHE is None:
        _NC_CACHE = _build_program()
    return _NC_CACHE


def run(fp32_pulse: np.ndarray, trace: bool = False):
    fp32_pulse = np.ascontiguousarray(np.asarray(fp32_pulse, dtype=np.float32))
    assert fp32_pulse.shape == (N_ROWS, 32), fp32_pulse.shape
    nc = _get_nc()
    shards = np.split(fp32_pulse, N_CORES, axis=0)
    in_maps = [{"x": s} for s in shards]
    res = run_bass_kernel_spmd(nc, in_maps, list(range(N_CORES)), trace=trace)
    out = np.concatenate([r["y"] for r in res.results], axis=0)
    return out.astype(np.float32, copy=False), res


def kernel(fp32_pulse: np.ndarray) -> np.ndarray:
    out, _ = run(fp32_pulse, trace=False)
    return out


# revision 6
# speedup vs baseline: 1.0433x; 1.0433x over previous
"""FP32 -> FP8 E4M3 bit-pulse converter on 8 Trainium2 NeuronCores.

Input : fp32_pulse [2097152, 32] float32 of 0/1 pulses, [S, E7..E0, M22..M0]
Output: [2097152, 8] float32 of 0/1 pulses, [S, E3..E0, M2..M0]

Strategy (per core, batch-sharded 8 ways):
  - SWDGE cast-DMA loads the fp32 pulses as uint8 (4x less SBUF traffic).
  - q = 32*exp + 16*m22 + 8*m21 + 4*m20 + 2*m19 + sticky  (13-bit int) via a
    scalar_tensor_tensor MAC chain; sticky = OR of the 19 low mantissa bits,
    computed as a segmented reduce_max over the int32-bitcast byte words.
  - v = bitcast(int32(max(q,3712) * 2^18)): exactly the fp32 with exponent
    field = exp, mantissa = m22..m19 | sticky<<18.  The underflow clamp maps
    exp<=116 to a value that rounds to zero in fp8.
  - Hardware fp32->fp8e4 copy performs the exact RNE (incl. subnormals).
  - Overflow (exp>=135) forces byte 0x7E (=15/6) per the reference.
  - 7 low bits of the byte + the sign pulse are scattered to the output.
"""

import numpy as np

import concourse.bass as bass
import concourse.bacc as bacc
import concourse.mybir as mybir
from concourse import tile
from concourse.bass_utils import run_bass_kernel_spmd

N_ROWS = 2097152
N_CORES = 8
ROWS_PER_CORE = N_ROWS // N_CORES          # 262144

F8 = 8192                                   # uint8 tile bytes per partition/chunk
ROWS_PER_CHUNK = 128 * F8 // 32             # 32768
N_CHUNKS = ROWS_PER_CORE // ROWS_PER_CHUNK  # 8
SEG = F8 // 32                              # segments (rows) per partition/chunk
FO = SEG * 8                                # out floats per partition/chunk

dt = mybir.dt
Alu = mybir.AluOpType

MAC_W = [float(2 ** (13 - i)) for i in range(1, 13)]  # col i weight, i=1..12


def _build_program(repeat: int = 1):
    nc = bacc.Bacc("TRN2", target_bir_lowering=False, debug=False,
                   num_devices=N_CORES)
    x_dram = nc.dram_tensor("x", [ROWS_PER_CORE, 32], dt.float32,
                            kind="ExternalInput")
    y_dram = nc.dram_tensor("y", [ROWS_PER_CORE, 8], dt.float32,
                            kind="ExternalOutput")
    x_ap = x_dram.ap().rearrange("(c p f) w -> c p (f w)", c=N_CHUNKS, p=128)
    y_ap = y_dram.ap().rearrange("(c p f) w -> c p (f w)", c=N_CHUNKS, p=128)

    with tile.TileContext(nc) as tc:
        with (
            tc.tile_pool(name="xin", bufs=3) as xin_pool,
            tc.tile_pool(name="out", bufs=3) as out_pool,
            tc.tile_pool(name="wrk", bufs=2) as wrk,
        ):
            for c in [c for _ in range(repeat) for c in range(N_CHUNKS)]:
                x8 = xin_pool.tile([128, F8], dt.uint8, tag="x8")
                nc.gpsimd.dma_start(x8[:], x_ap[c])

                x8v = x8[:].rearrange("p (s c) -> p s c", c=32)
                xw3d = x8[:].bitcast(dt.int32).rearrange("p (s w) -> p s w", w=8)

                # sticky = OR of cols 13..31 = (max(words 4..7) | (word3>=256)) > 0
                red = wrk.tile([128, SEG], dt.float32, tag="red")
                nc.vector.tensor_reduce(red[:], xw3d[:, :, 4:8],
                                        axis=mybir.AxisListType.X, op=Alu.max)
                sor = wrk.tile([128, SEG], dt.float32, tag="sor")
                nc.vector.scalar_tensor_tensor(sor[:], xw3d[:, :, 3], 256.0,
                                               red[:], op0=Alu.is_ge, op1=Alu.max)

                # q MAC chain over cols 1..12, then fold sticky
                qa = wrk.tile([128, SEG], dt.float32, tag="qa")
                qb = wrk.tile([128, SEG], dt.float32, tag="qb")
                nc.vector.tensor_scalar(qa[:], x8v[:, :, 1], MAC_W[0], None,
                                        op0=Alu.mult)
                cur, nxt = qa, qb
                for i in range(2, 13):
                    nc.vector.scalar_tensor_tensor(nxt[:], x8v[:, :, i],
                                                   MAC_W[i - 1], cur[:],
                                                   op0=Alu.mult, op1=Alu.add)
                    cur, nxt = nxt, cur
                q = nxt
                nc.vector.scalar_tensor_tensor(q[:], sor[:], 0.0, cur[:],
                                               op0=Alu.is_gt, op1=Alu.add)

                # v bits = int32(max(q, 3712) * 2^18); fp8 cast on ACT
                vb = wrk.tile([128, SEG], dt.int32, tag="vb")
                nc.vector.tensor_scalar(vb[:], q[:], 3712.0, 262144.0,
                                        op0=Alu.max, op1=Alu.mult)
                f8 = wrk.tile([128, SEG], dt.float8e4, tag="f8")
                nc.scalar.copy(f8[:], vb[:].bitcast(dt.float32))
                u = wrk.tile([128, SEG], dt.float32, tag="u")
                nc.scalar.copy(u[:], f8[:].bitcast(dt.uint8))

                # overflow select: uf = u + (q>=4320)*(126-u)
                ovf = wrk.tile([128, SEG], dt.float32, tag="ovf")
                nc.gpsimd.tensor_scalar(ovf[:], q[:], 4320.0, None, op0=Alu.is_ge)
                d_t = wrk.tile([128, SEG], dt.float32, tag="d")
                nc.gpsimd.tensor_scalar(d_t[:], u[:], -1.0, 126.0,
                                        op0=Alu.mult, op1=Alu.add)
                m_t = wrk.tile([128, SEG], dt.float32, tag="m")
                nc.gpsimd.tensor_tensor(m_t[:], d_t[:], ovf[:], op=Alu.mult)
                uf = wrk.tile([128, SEG], dt.float32, tag="uf")
                nc.gpsimd.tensor_tensor(uf[:], m_t[:], u[:], op=Alu.add)
                ui = wrk.tile([128, SEG], dt.int32, tag="ui")
                nc.scalar.copy(ui[:], uf[:])

                o_i = wrk.tile([128, FO], dt.int32, tag="oi")
                oi3d = o_i[:].rearrange("p (s c) -> p s c", c=8)
                nc.scalar.copy(oi3d[:, :, 0], x8v[:, :, 0])      # sign
                for j in range(1, 8):
                    nc.vector.tensor_scalar(oi3d[:, :, j], ui[:], 7 - j, 1,
                                            op0=Alu.logical_shift_right,
                                            op1=Alu.bitwise_and)
                o_t = out_pool.tile([128, FO], dt.float32, tag="o")
                nc.scalar.copy(o_t[:], o_i[:])

                nc.sync.dma_start(y_ap[c], o_t[:])

    nc.compile()
    return nc


_NC_CACHE = {}


def _get_nc(repeat: int = 1):
    if repeat not in _NC_CACHE:
        _NC_CACHE[repeat] = _build_program(repeat)
    return _NC_CACHE[repeat]


def run(fp32_pulse: np.ndarray, trace: bool = False):
    fp32_pulse = np.ascontiguousarray(np.asarray(fp32_pulse, dtype=np.float32))
    assert fp32_pulse.shape == (N_ROWS, 32), fp32_pulse.shape
    nc = _get_nc()
    shards = np.split(fp32_pulse, N_CORES, axis=0)
    in_maps = [{"x": s} for s in shards]
    res = run_bass_kernel_spmd(nc, in_maps, list(range(N_CORES)), trace=trace)
    out = np.concatenate([r["y"] for r in res.results], axis=0)
    return out.astype(np.float32, copy=False), res


def kernel(fp32_pulse: np.ndarray) -> np.ndarray:
    out, _ = run(fp32_pulse, trace=False)
    return out
